# revision 6
# baseline (speedup 1.0000x reference)
"""Trainium2 Bass kernel for nn_Attention_85186381348942.

2D self-attention block: x [2, 512, 64, 64], 8 heads x 64 dim, n = 4096 tokens.
  qkv = w_qkv @ x ; per head: S = (q*scale)^T k ; P = exp(S) (softmax without
  max-subtraction -- logits are small); out = (P/Z) @ v ; y = w_out @ out + b.

Sharding: 8 cores = (batch b in {0,1}) x (head-pair hp in {0..3}); each core
computes 2 heads of one batch and the partial output projection for its head
slice. Host sums the 4 partials per batch and adds bias.

Device-side schedule (ACT-exp is the bottleneck engine; everything else is
scheduled to keep it 100% busy):
 - scores are computed TRANSPOSED: S_T[j, i] = k^T q so the softmax'd tile
   P [j, i] feeds the PV matmul directly as the moving operand.
 - v' carries a "ones" column so one PV output row is the softmax denominator
   Z[i]. Head A occupies PSUM rows 0..64 (Z at 64) of its accumulator; head B
   is shifted to rows 63..127 (Z at 63) so the two normalized halves pack into
   one [128, WI] tile and the output projection runs K=128 (4 matmuls/ib
   instead of 8).
 - k/v (and re-computed q) for iteration r+1 are produced DURING iteration r,
   spread evenly (one 4-chunk matmul item per ~5 steps) into rep-parity
   double buffers, so no i-block is PE-oversubscribed.
 - epilogue (1/Z, normalize, projection) of i-block ib runs during ib+1 at
   fixed j-slots. Projection + producer matmuls borrow the idle out-PSUM
   parity banks (freed at j=9 by the normalize, reused at the next-next ib's
   j=0), so they never steal the scores pool's double buffer.
 - 1/Z broadcast across partitions goes SBUF->DRAM->broadcast-read on the
   gpsimd (SWDGE) queue, keeping the sync queue free for output stores.
"""

import numpy as np
import ml_dtypes

import concourse.bass as bass
import concourse.tile as tile
from concourse import bacc, mybir
from concourse.bass_utils import run_bass_kernel_spmd

BF16 = mybir.dt.bfloat16
F32 = mybir.dt.float32
AF = mybir.ActivationFunctionType

HEADS = 8
DIM_HEAD = 64
DIM = 512
N = 4096  # 64*64 tokens
N_CORES = 8
NB = 8  # number of 512-wide i-blocks
JB = 32  # number of 128-wide j-blocks
WI = 512  # i-block width


def build_program(repeats: int = 1, small_out: bool = False) -> bass.Bass:
    nc = bacc.Bacc(None, target_bir_lowering=False, num_devices=N_CORES)

    x = nc.dram_tensor("x", [DIM, N], BF16, kind="ExternalInput")
    wqk = nc.dram_tensor("wqk", [DIM, 256], BF16, kind="ExternalInput")
    wv = nc.dram_tensor("wv", [DIM, 128], BF16, kind="ExternalInput")
    wo = nc.dram_tensor("wo", [128, 4, 128], BF16, kind="ExternalInput")
    if small_out:
        out = nc.dram_tensor("out", [4, 128, WI], F32, kind="ExternalOutput")
        out_r = out
    else:
        out = nc.dram_tensor("out", [DIM, N], F32, kind="ExternalOutput")
        out_r = out.rearrange("(o p) n -> o p n", p=128)

    x_r = x.rearrange("(o p) n -> p o n", p=128)
    wqk_r = wqk.rearrange("(o p) m -> p o m", p=128)
    wv_r = wv.rearrange("(o p) m -> p o m", p=128)

    TOT = repeats * NB * JB

    with tile.TileContext(nc) as tc:
        with (
            tc.tile_pool(name="singles", bufs=1) as singles,
            tc.tile_pool(name="pp", bufs=4) as pp,  # P = exp(S) tiles
            tc.tile_pool(name="sbsm", bufs=2) as sbsm,  # small sbuf temps
            tc.tile_pool(name="stg", bufs=4) as stg,  # output staging
            tc.tile_pool(name="ps_s", bufs=2, space="PSUM") as ps_s,  # scores
            tc.tile_pool(name="ps_o", bufs=1, space="PSUM") as ps_o,  # PV+borrow
            tc.tile_pool(name="dram", bufs=2, space="DRAM") as dram,
        ):
            # ---- input DMAs: x per n-block on HWDGE (sync); weights on SWDGE
            x_sb = singles.tile([128, 4, N], BF16)
            for nb0 in range(NB):
                nc.sync.dma_start(
                    x_sb[:, :, nb0 * WI:(nb0 + 1) * WI],
                    x_r[:, :, nb0 * WI:(nb0 + 1) * WI])
            wqk_sb = singles.tile([128, 4, 256], BF16)
            nc.gpsimd.dma_start(wqk_sb[:], wqk_r)
            wv_sb = singles.tile([128, 4, 128], BF16)
            nc.gpsimd.dma_start(wv_sb[:], wv_r)
            wo_sb = singles.tile([128, 4, 128], BF16)
            nc.gpsimd.dma_start(wo_sb[:], wo[:])

            q_sb = singles.tile([128, N], BF16)
            # k and v' double-buffered by rep parity. v' per head:
            # [j-part, parity, chunk, 65]; head A: cols 0-63 dims, col 64 ones
            # (-> Z_A at out row 64); head B: col 0 ones, cols 1-64 dims
            # (-> Z_B at out row 63, dims at rows 64-127).
            k_sb = singles.tile([128, 2, N], BF16)
            vA = singles.tile([128, 2, JB, 65], BF16)
            vB = singles.tile([128, 2, JB, 65], BF16)
            nc.vector.memset(vA[:], 1.0)
            nc.vector.memset(vB[:], 1.0)
            # dummy activation: pull the ACT table load into the DMA window
            warm = singles.tile([1, 8], F32)
            nc.vector.memset(warm[:], 0.0)
            nc.scalar.activation(warm[:], warm[:], AF.Exp)

            # ---- borrowed-PSUM helper ---------------------------------------
            # During i-block gb, the out-psum parity pair (1 - gb%2) is idle
            # from j=9 (normalize of epi[gb-1] done) until the next-next
            # alloc_outs -- projection and producer matmuls borrow those two
            # banks, alternating between them. The prologue (no accumulators
            # live yet) round-robins over all four.
            borrow_rr = {"base": 0, "mod": 4, "cnt": 0}

            def set_borrow(base, mod):
                borrow_rr.update(base=base, mod=mod, cnt=0)

            def borrow_tile(shape, name):
                tags = ("oA0", "oB0", "oA1", "oB1")
                i = borrow_rr["base"] + borrow_rr["cnt"] % borrow_rr["mod"]
                borrow_rr["cnt"] += 1
                return ps_o.tile(shape, F32, tag=tags[i], name=name)

            def emit_prod_mms(item, par, ps, chunks):
                """Chunk matmuls for one producer item into psum tile ps."""
                kind, idx = item
                for o in chunks:
                    if kind == "k":
                        nc.tensor.matmul(
                            ps[:], lhsT=wqk_sb[:, o, 128:256],
                            rhs=x_sb[:, o, idx * WI:(idx + 1) * WI],
                            start=(o == 0), stop=(o == 3),
                            skip_group_check=True)
                    elif kind == "q":
                        nc.tensor.matmul(
                            ps[:], lhsT=wqk_sb[:, o, 0:128],
                            rhs=x_sb[:, o, idx * WI:(idx + 1) * WI],
                            start=(o == 0), stop=(o == 3),
                            skip_group_check=True)
                    else:  # v chunk: out [n-chunk 128, 128 = dA|dB]
                        nc.tensor.matmul(
                            ps[:, 0:128],
                            lhsT=x_sb[:, o, idx * 128:(idx + 1) * 128],
                            rhs=wv_sb[:, o, :],
                            start=(o == 0), stop=(o == 3),
                            skip_group_check=True)

            def emit_prod_copy(item, par, ps):
                kind, idx = item
                if kind == "k":
                    nc.vector.tensor_copy(
                        k_sb[:, par, idx * WI:(idx + 1) * WI], ps[:])
                elif kind == "q":
                    nc.vector.tensor_copy(
                        q_sb[:, idx * WI:(idx + 1) * WI], ps[:])
                else:
                    nc.vector.tensor_copy(vA[:, par, idx, 0:64], ps[:, 0:64])
                    nc.vector.tensor_copy(vB[:, par, idx, 1:65], ps[:, 64:128])

            # ---- prologue: produce q, k(par0), v(par0) for rep 0 ------------
            for nb in range(NB):
                for it in ([("k", nb), ("q", nb)] +
                           [("v", 4 * nb + c) for c in range(4)]):
                    shape = [128, WI] if it[0] in ("k", "q") else [128, 128]
                    ps = borrow_tile(shape, "prod_ps")
                    emit_prod_mms(it, 0, ps, range(4))
                    emit_prod_copy(it, 0, ps)

            # ---- main loop --------------------------------------------------
            def emit_scores(rep, ib, j):
                par = rep % 2
                i0 = ib * WI
                j0 = j * 128
                s_ps = ps_s.tile([128, 2 * WI], F32, tag="s", name="s_ps")
                nc.tensor.matmul(
                    s_ps[:, 0:WI],
                    lhsT=k_sb[0:64, par, j0:j0 + 128],
                    rhs=q_sb[0:64, i0:i0 + WI],
                    start=True, stop=True)
                nc.tensor.matmul(
                    s_ps[:, WI:2 * WI],
                    lhsT=k_sb[64:128, par, j0:j0 + 128],
                    rhs=q_sb[64:128, i0:i0 + WI],
                    start=True, stop=True)
                return s_ps

            def emit_epilogue_piece(ctx, piece):
                ib = ctx["ib"]
                i0 = ib * WI
                o_A, o_B = ctx["out_ps"]
                if piece == 0:  # reciprocal of Z rows (DVE)
                    zrA = sbsm.tile([65, WI], F32, tag="zrA", name="zrA")
                    zrB = sbsm.tile([64, WI], F32, tag="zrB", name="zrB")
                    nc.vector.reciprocal(zrA[64:65, :], o_A[64:65, :])
                    nc.vector.reciprocal(zrB[63:64, :], o_B[63:64, :])
                    ctx["zrA"], ctx["zrB"] = zrA, zrB
                elif piece == 1:  # stage 1/Z to DRAM (gpsimd queue)
                    zdA = dram.tile([1, WI], F32, tag="zdA", name="zdA")
                    zdB = dram.tile([1, WI], F32, tag="zdB", name="zdB")
                    nc.gpsimd.dma_start(zdA[:], ctx["zrA"][64:65, :])
                    nc.gpsimd.dma_start(zdB[:], ctx["zrB"][63:64, :])
                    ctx["zdA"], ctx["zdB"] = zdA, zdB
                elif piece == 2:  # broadcast across partitions (gpsimd queue)
                    rsb = sbsm.tile([128, WI], F32, tag="rsb", name="rsb")
                    nc.gpsimd.dma_start(rsb[0:64, :],
                                        ctx["zdA"].to_broadcast([64, WI]))
                    nc.gpsimd.dma_start(rsb[64:128, :],
                                        ctx["zdB"].to_broadcast([64, WI]))
                    ctx["rsb"] = rsb
                elif piece == 3:  # normalize into packed [128, WI] (DVE)
                    on = sbsm.tile([128, WI], BF16, tag="on", name="on")
                    nc.vector.tensor_mul(on[0:64, :], o_A[0:64, :],
                                         ctx["rsb"][0:64, :])
                    nc.vector.tensor_mul(on[64:128, :], o_B[64:128, :],
                                         ctx["rsb"][64:128, :])
                    ctx["on"] = on
                else:  # pieces 4..7: packed projection chunk + store
                    o = piece - 4
                    pr = borrow_tile([128, WI], "pr_ps")
                    nc.tensor.matmul(
                        pr[:], lhsT=wo_sb[:, o, :], rhs=ctx["on"][:],
                        start=True, stop=True, skip_group_check=True)
                    st = stg.tile([128, WI], F32, tag="st", name="st")
                    nc.vector.tensor_copy(st[:], pr[:])
                    if small_out:
                        nc.sync.dma_start(out_r[o, :, :], st[:])
                    else:
                        nc.sync.dma_start(out_r[o, :, i0:i0 + WI], st[:])

            # epilogue(prev ib) piece -> j-slot within the current i-block
            PIECE_AT = {1: 0, 2: 1, 4: 2, 9: 3, 11: 4, 14: 5, 17: 6, 20: 7}
            # producer item start slots within the current i-block (2 chunk
            # matmuls at j, 2 at j+1; disjoint from the projection j-slots)
            PROD_AT = {12: 0, 15: 1, 18: 2, 21: 3, 24: 4, 27: 5}

            epi = {}
            s_tiles = {}
            outs = {}

            def alloc_outs(g):
                ib = (g // JB) % NB
                par = (g // JB) % 2
                o_A = ps_o.tile([128, WI], F32, tag=f"oA{par}", name="out_A")
                o_B = ps_o.tile([128, WI], F32, tag=f"oB{par}", name="out_B")
                outs[g // JB] = (o_A, o_B)
                epi[g // JB] = {"ib": ib, "out_ps": (o_A, o_B)}

            alloc_outs(0)
            s_tiles[0] = emit_scores(0, 0, 0)
            prod_items = {}  # (gb, slot) -> (item, psum tile)

            for g in range(TOT):
                gb, j = divmod(g, JB)  # global i-block counter, j-step
                rep, ib = divmod(gb, NB)
                par = rep % 2
                if j == 0:
                    set_borrow(2 * (1 - gb % 2), 2)
                out_A, out_B = outs[gb]
                if g + 1 < TOT:
                    ngb, nj = divmod(g + 1, JB)
                    nrep, nib = divmod(ngb, NB)
                    if nj == 0:
                        alloc_outs(g + 1)
                    s_tiles[g + 1] = emit_scores(nrep, nib, nj)
                s_ps = s_tiles.pop(g)
                p_sb = pp.tile([128, 2 * WI], BF16, tag="p", name="p_sb")
                nc.scalar.activation(p_sb[:], s_ps[:], AF.Exp)
                nc.tensor.matmul(
                    out_A[0:65, :], lhsT=vA[:, par, j, :], rhs=p_sb[:, 0:WI],
                    start=(j == 0), stop=(j == JB - 1),
                    skip_group_check=True)
                nc.tensor.matmul(
                    out_B[63:128, :], lhsT=vB[:, par, j, :],
                    rhs=p_sb[:, WI:2 * WI],
                    start=(j == 0), stop=(j == JB - 1),
                    skip_group_check=True)
                prev = epi.get(gb - 1)
                if prev is not None and j in PIECE_AT:
                    emit_epilogue_piece(prev, PIECE_AT[j])
                # producer items for rep+1 (k/v into parity 1-par; q rewrite):
                # during ib, item list [k_ib, q_{ib-1}, v_{4ib..4ib+3}]
                if rep + 1 < repeats:
                    if j in PROD_AT:
                        slot = PROD_AT[j]
                        items = ([("k", ib), ("q", (ib - 1) % NB)] +
                                 [("v", 4 * ib + c) for c in range(4)])
                        it = items[slot]
                        shape = [128, WI] if it[0] in ("k", "q") else [128, 128]
                        ps = borrow_tile(shape, "prod_ps")
                        emit_prod_mms(it, 1 - par, ps, range(2))
                        prod_items[(gb, slot)] = (it, ps)
                    if (j - 1) in PROD_AT:
                        it, ps = prod_items.pop((gb, PROD_AT[j - 1]))
                        emit_prod_mms(it, 1 - par, ps, range(2, 4))
                        emit_prod_copy(it, 1 - par, ps)

            set_borrow(2 * (1 - (repeats * NB - 1) % 2), 2)
            for piece in range(8):
                emit_epilogue_piece(epi[repeats * NB - 1], piece)

    nc.finalize()
    return nc


_PROGRAM_CACHE = {}


def _get_program(**kw) -> bass.Bass:
    key = tuple(sorted(kw.items()))
    if key not in _PROGRAM_CACHE:
        _PROGRAM_CACHE[key] = build_program(**kw)
    return _PROGRAM_CACHE[key]


def _prep_inputs(x, w_qkv, w_out):
    """Build the per-core input maps (all bf16 host-side casts)."""
    scale = DIM_HEAD ** -0.5
    xb = x.reshape(2, DIM, N)
    in_maps = []
    for core in range(N_CORES):
        b, hp = divmod(core, 4)
        r0 = hp * 128
        wq = w_qkv[r0:r0 + 128] * scale          # [128, 512]
        wk = w_qkv[DIM + r0:DIM + r0 + 128]      # [128, 512]
        wvr = w_qkv[2 * DIM + r0:2 * DIM + r0 + 128]
        wqk_c = np.concatenate([wq.T, wk.T], axis=1)   # [512, 256]
        wv_t = wvr.T                                   # [512, 128]
        wo_p = np.ascontiguousarray(
            w_out[:, r0:r0 + 128].T).reshape(128, 4, 128)
        in_maps.append({
            "x": xb[b].astype(ml_dtypes.bfloat16),
            "wqk": wqk_c.astype(ml_dtypes.bfloat16),
            "wv": wv_t.astype(ml_dtypes.bfloat16),
            "wo": wo_p.astype(ml_dtypes.bfloat16),
        })
    return in_maps


def _run(nc, in_maps):
    try:
        return run_bass_kernel_spmd(nc, in_maps, core_ids=list(range(N_CORES)))
    except Exception:
        # one retry: a previously-wedged device surfaces as a transient
        # NRT_EXEC_UNIT_UNRECOVERABLE on the first execution
        return run_bass_kernel_spmd(nc, in_maps, core_ids=list(range(N_CORES)))


def kernel(x, w_qkv, w_out, b_out):
    nc = _get_program()
    in_maps = _prep_inputs(np.asarray(x), np.asarray(w_qkv), np.asarray(w_out))
    res = _run(nc, in_maps)
    partials = np.stack([r["out"] for r in res.results])  # [8, 512, 4096]
    y = partials.reshape(2, 4, DIM, N).sum(axis=1)
    y += np.asarray(b_out)[None, :, None]
    return y.reshape(2, DIM, 64, 64).astype(np.float32)


# revision 21
# speedup vs baseline: 1.6844x; 1.6844x over previous
"""Trainium2 Bass kernel for nn_Attention_85186381348942.

2D self-attention block: x [2, 512, 64, 64], 8 heads x 64 dim, n = 4096 tokens.
  qkv = w_qkv @ x ; per head: S = (q*scale)^T k ; P = exp(S) (softmax without
  max-subtraction -- logits are small); out = (P/Z) @ v ; y = w_out @ out + b.

Sharding: 8 cores = (batch b in {0,1}) x (head-pair hp in {0..3}); each core
computes 2 heads of one batch and the partial output projection for its head
slice. Host sums the 4 partials per batch and adds bias.

Device-side schedule (ACT-exp is the bottleneck engine; everything else is
scheduled to keep it 100% busy):
 - scores are computed TRANSPOSED: S_T[j, i] = k^T q so the softmax'd tile
   P [j, i] feeds the PV matmul directly as the moving operand.
 - v' carries a "ones" column so one PV output row is the softmax denominator
   Z[i]. Head A occupies PSUM rows 0..64 (Z at 64) of its accumulator; head B
   is shifted to rows 63..127 (Z at 63) so the two normalized halves pack into
   one [128, WI] tile and the output projection runs K=128 (4 matmuls/ib
   instead of 8).
 - k/v (and re-computed q) for iteration r+1 are produced DURING iteration r,
   spread evenly (one 4-chunk matmul item per ~5 steps) into rep-parity
   double buffers, so no i-block is PE-oversubscribed.
 - epilogue (1/Z, normalize, projection) of i-block ib runs during ib+1 at
   fixed j-slots. Projection + producer matmuls borrow the idle out-PSUM
   parity banks (freed at j=9 by the normalize, reused at the next-next ib's
   j=0), so they never steal the scores pool's double buffer.
 - 1/Z broadcast across partitions goes SBUF->DRAM->broadcast-read on the
   gpsimd (SWDGE) queue, keeping the sync queue free for output stores.
"""

import numpy as np
import ml_dtypes

import concourse.bass as bass
import concourse.tile as tile
from concourse import bacc, mybir
from concourse.bass_utils import run_bass_kernel_spmd

BF16 = mybir.dt.bfloat16
F32 = mybir.dt.float32
AF = mybir.ActivationFunctionType

HEADS = 8
DIM_HEAD = 64
DIM = 512
N = 4096  # 64*64 tokens
N_CORES = 8
NB = 8  # number of 512-wide i-blocks
JB = 32  # number of 128-wide j-blocks
WI = 512  # i-block width


def build_program(repeats: int = 1, small_out: bool = False) -> bass.Bass:
    nc = bacc.Bacc(None, target_bir_lowering=False, num_devices=N_CORES)

    x = nc.dram_tensor("x", [DIM, N], BF16, kind="ExternalInput")
    wqk = nc.dram_tensor("wqk", [DIM, 256], BF16, kind="ExternalInput")
    wv = nc.dram_tensor("wv", [DIM, 128], BF16, kind="ExternalInput")
    wo = nc.dram_tensor("wo", [64, 2, DIM], BF16, kind="ExternalInput")
    if small_out:
        out = nc.dram_tensor("out", [4, 128, WI], F32, kind="ExternalOutput")
        out_r = out
    else:
        out = nc.dram_tensor("out", [DIM, N], F32, kind="ExternalOutput")
        out_r = out.rearrange("(o p) n -> o p n", p=128)

    x_r = x.rearrange("(o p) n -> p o n", p=128)
    wqk_r = wqk.rearrange("(o p) m -> p o m", p=128)
    wv_r = wv.rearrange("(o p) m -> p o m", p=128)

    TOT = repeats * NB * JB

    with tile.TileContext(nc) as tc:
        with (
            tc.tile_pool(name="singles", bufs=1) as singles,
            tc.tile_pool(name="pp", bufs=6) as pp,  # P = exp(S) tiles
            tc.tile_pool(name="sbsm", bufs=2) as sbsm,  # small sbuf temps
            tc.tile_pool(name="stg", bufs=4) as stg,  # output staging
            tc.tile_pool(name="ps_s", bufs=2, space="PSUM") as ps_s,  # scores
            tc.tile_pool(name="ps_o", bufs=1, space="PSUM") as ps_o,  # PV+borrow
            tc.tile_pool(name="dram", bufs=2, space="DRAM") as dram,
        ):
            # ---- input DMAs: x per n-block on HWDGE (sync); weights on SWDGE
            x_sb = singles.tile([128, 4, N], BF16)
            for nb0 in range(NB):
                nc.sync.dma_start(
                    x_sb[:, :, nb0 * WI:(nb0 + 1) * WI],
                    x_r[:, :, nb0 * WI:(nb0 + 1) * WI])
            wqk_sb = singles.tile([128, 4, 256], BF16)
            nc.gpsimd.dma_start(wqk_sb[:], wqk_r)
            wv_sb = singles.tile([128, 4, 128], BF16)
            nc.gpsimd.dma_start(wv_sb[:], wv_r)
            wo_sb = singles.tile([64, 2, DIM], BF16)
            nc.gpsimd.dma_start(wo_sb[:], wo[:])

            q_sb = singles.tile([128, N], BF16)
            # k and v' double-buffered by rep parity. v' per head:
            # [j-part, parity, chunk, 65]; cols 0-63 dims, col 64 ones
            # (-> softmax denominator Z at out row 64).
            k_sb = singles.tile([128, 2, N], BF16)
            vA = singles.tile([128, 2, JB, 65], BF16)
            vB = singles.tile([128, 2, JB, 65], BF16)
            nc.vector.memset(vA[:], 1.0)
            nc.vector.memset(vB[:], 1.0)
            # dummy activation: pull the ACT table load into the DMA window
            warm = singles.tile([1, 8], F32)
            nc.vector.memset(warm[:], 0.0)
            nc.scalar.activation(warm[:], warm[:], AF.Exp)

            # ---- borrowed-PSUM helper ---------------------------------------
            # During i-block gb, the out-psum parity pair (1 - gb%2) is idle
            # from j=9 (normalize of epi[gb-1] done) until the next-next
            # alloc_outs -- projection and producer matmuls borrow those two
            # banks per the explicit tag plan in PIECE_TAG/PROD_AT. The
            # prologue (no accumulators live yet) round-robins over all four.
            borrow_rr = {"cnt": 0}

            def borrow_tile(shape, name):
                tags = ("oA0", "oB0", "oA1", "oB1")
                i = borrow_rr["cnt"] % 4
                borrow_rr["cnt"] += 1
                return ps_o.tile(shape, F32, tag=tags[i], name=name)

            def emit_prod_mms(item, par, ps, chunks):
                """Chunk matmuls for one producer item into psum tile ps."""
                kind, idx = item
                for o in chunks:
                    if kind == "k":
                        nc.tensor.matmul(
                            ps[:], lhsT=wqk_sb[:, o, 128:256],
                            rhs=x_sb[:, o, idx * WI:(idx + 1) * WI],
                            start=(o == 0), stop=(o == 3),
                            skip_group_check=True)
                    elif kind == "q":
                        nc.tensor.matmul(
                            ps[:], lhsT=wqk_sb[:, o, 0:128],
                            rhs=x_sb[:, o, idx * WI:(idx + 1) * WI],
                            start=(o == 0), stop=(o == 3),
                            skip_group_check=True)
                    else:  # v chunk: out [n-chunk 128, 128 = dA|dB]
                        nc.tensor.matmul(
                            ps[:, 0:128],
                            lhsT=x_sb[:, o, idx * 128:(idx + 1) * 128],
                            rhs=wv_sb[:, o, :],
                            start=(o == 0), stop=(o == 3),
                            skip_group_check=True)

            def emit_prod_copy(item, par, ps):
                kind, idx = item
                if kind == "k":
                    nc.vector.tensor_copy(
                        k_sb[:, par, idx * WI:(idx + 1) * WI], ps[:])
                elif kind == "q":
                    nc.vector.tensor_copy(
                        q_sb[:, idx * WI:(idx + 1) * WI], ps[:])
                else:
                    nc.vector.tensor_copy(vA[:, par, idx, 0:64], ps[:, 0:64])
                    nc.vector.tensor_copy(vB[:, par, idx, 0:64], ps[:, 64:128])

            # ---- prologue: produce q, k(par0), v(par0) for rep 0 ------------
            for nb in range(NB):
                for it in ([("k", nb), ("q", nb)] +
                           [("v", 4 * nb + c) for c in range(4)]):
                    shape = [128, WI] if it[0] in ("k", "q") else [128, 128]
                    ps = borrow_tile(shape, "prod_ps")
                    emit_prod_mms(it, 0, ps, range(4))
                    emit_prod_copy(it, 0, ps)

            # ---- main loop --------------------------------------------------
            def emit_scores(rep, ib, j):
                par = rep % 2
                i0 = ib * WI
                j0 = j * 128
                s_ps = ps_s.tile([128, 2 * WI], F32, tag="s", name="s_ps")
                nc.tensor.matmul(
                    s_ps[:, 0:WI],
                    lhsT=k_sb[0:64, par, j0:j0 + 128],
                    rhs=q_sb[0:64, i0:i0 + WI],
                    start=True, stop=True)
                nc.tensor.matmul(
                    s_ps[:, WI:2 * WI],
                    lhsT=k_sb[64:128, par, j0:j0 + 128],
                    rhs=q_sb[64:128, i0:i0 + WI],
                    start=True, stop=True)
                return s_ps

            def emit_epilogue_piece(ctx, piece, fp=None):
                ib = ctx["ib"]
                i0 = ib * WI
                o_A, o_B = ctx["out_ps"]
                if piece == 0:  # reciprocal of Z rows (DVE)
                    for h, o_ps in ((0, o_A), (1, o_B)):
                        zr = sbsm.tile([65, WI], F32, tag=f"zr{h}", name="zr")
                        nc.vector.reciprocal(zr[64:65, :], o_ps[64:65, :])
                        ctx[f"zr{h}"] = zr
                elif piece == 1:  # stage 1/Z to DRAM (gpsimd queue)
                    for h in (0, 1):
                        zd = dram.tile([1, WI], F32, tag=f"zd{h}", name="zd")
                        nc.gpsimd.dma_start(zd[:], ctx[f"zr{h}"][64:65, :])
                        ctx[f"zd{h}"] = zd
                elif piece == 2:  # broadcast across partitions (gpsimd queue)
                    for h in (0, 1):
                        rs = sbsm.tile([64, WI], F32, tag=f"rs{h}", name="rs")
                        nc.gpsimd.dma_start(rs[:],
                                            ctx[f"zd{h}"].to_broadcast([64, WI]))
                        ctx[f"rs{h}"] = rs
                elif piece == 3:  # normalize (DVE)
                    for h, o_ps in ((0, o_A), (1, o_B)):
                        on = sbsm.tile([64, WI], BF16, tag=f"on{h}", name="on")
                        nc.vector.tensor_mul(on[:], o_ps[0:64, :],
                                             ctx[f"rs{h}"][:])
                        ctx[f"on{h}"] = on
                else:  # pieces 4..7: projection chunk (two K=64 halves) + store
                    o = piece - 4
                    if fp is None:
                        pr = borrow_tile([128, WI], "pr_ps")
                    else:
                        pr = ps_o.tile([128, WI], F32,
                                       tag=f"o{PIECE_TAG[piece]}{fp}",
                                       name="pr_ps")
                    for h in (0, 1):
                        nc.tensor.matmul(
                            pr[:],
                            lhsT=wo_sb[:, h, o * 128:(o + 1) * 128],
                            rhs=ctx[f"on{h}"][:],
                            start=(h == 0), stop=(h == 1),
                            skip_group_check=True)
                    st = stg.tile([128, WI], F32, tag="st", name="st")
                    nc.vector.tensor_copy(st[:], pr[:])
                    if small_out:
                        nc.sync.dma_start(out_r[o, :, :], st[:])
                    else:
                        nc.sync.dma_start(out_r[o, :, i0:i0 + WI], st[:])

            # Per-j action schedule within each i-block. Every step in
            # j=10..29 carries at most +213ns of extra PE work (one producer
            # contraction chunk, one v-chunk item, or one projection pair) so
            # the per-step PE total never exceeds the ACT step time.
            # Borrow banks: 'A'/'B' = the two out-psum banks of the free
            # parity; same-tag users are spaced so each one's DVE copy lands
            # before the next allocation.
            PIECE_AT = {1: 0, 2: 1, 4: 2, 9: 3, 11: 4, 13: 5, 15: 6, 17: 7}
            PIECE_TAG = {4: "B", 5: "A", 6: "B", 7: "A"}
            # producer actions: ('k'|'q', chunk) one contraction chunk per
            # step; ('v', i) one whole v-item (4 x 128-col matmuls) per step
            PROD_AT = {
                10: ("v", 3, "A"),
                19: ("k", 0, "B"), 21: ("k", 1, "B"),
                23: ("k", 2, "B"), 25: ("k", 3, "B"),
                20: ("q", 0, "A"), 22: ("q", 1, "A"),
                24: ("q", 2, "A"), 26: ("q", 3, "A"),
                27: ("v", 0, "B"), 28: ("v", 1, "A"), 29: ("v", 2, "B"),
            }

            epi = {}
            s_tiles = {}
            outs = {}

            def alloc_outs(g):
                ib = (g // JB) % NB
                par = (g // JB) % 2
                o_A = ps_o.tile([128, WI], F32, tag=f"oA{par}", name="out_A")
                o_B = ps_o.tile([128, WI], F32, tag=f"oB{par}", name="out_B")
                outs[g // JB] = (o_A, o_B)
                epi[g // JB] = {"ib": ib, "out_ps": (o_A, o_B)}

            alloc_outs(0)
            s_tiles[0] = emit_scores(0, 0, 0)
            prod_items = {}  # (gb, slot) -> (item, psum tile)

            for g in range(TOT):
                gb, j = divmod(g, JB)  # global i-block counter, j-step
                rep, ib = divmod(gb, NB)
                par = rep % 2
                out_A, out_B = outs[gb]
                if g + 1 < TOT:
                    ngb, nj = divmod(g + 1, JB)
                    nrep, nib = divmod(ngb, NB)
                    if nj == 0:
                        alloc_outs(g + 1)
                    s_tiles[g + 1] = emit_scores(nrep, nib, nj)
                s_ps = s_tiles.pop(g)
                p_sb = pp.tile([128, 2 * WI], BF16, tag="p", name="p_sb")
                nc.scalar.activation(p_sb[:], s_ps[:], AF.Exp)
                # ACT-independent PE work (projection pieces, producer chunks)
                # is emitted BEFORE the PV pair: PV(g) stalls on exp(g), and
                # anything queued behind it would otherwise delay the next
                # step's scores past the ACT deadline.
                prev = epi.get(gb - 1)
                fp = 1 - gb % 2
                if prev is not None and j in PIECE_AT:
                    emit_epilogue_piece(prev, PIECE_AT[j], fp)
                # producer actions for rep+1 (k/v into parity 1-par; q rewrite
                # of block ib-1, which this rep is done reading)
                if rep + 1 < repeats and j in PROD_AT:
                    kind, idx, tg = PROD_AT[j]
                    tag = f"o{tg}{fp}"
                    if kind == "v":
                        it = ("v", 4 * ib + idx)
                        ps = ps_o.tile([128, 128], F32, tag=tag, name="prod_ps")
                        emit_prod_mms(it, 1 - par, ps, range(4))
                        emit_prod_copy(it, 1 - par, ps)
                    else:
                        it = ("k", ib) if kind == "k" else ("q", (ib - 1) % NB)
                        if idx == 0:
                            prod_items[(gb, kind)] = ps_o.tile(
                                [128, WI], F32, tag=tag, name="prod_ps")
                        ps = prod_items[(gb, kind)]
                        emit_prod_mms(it, 1 - par, ps, [idx])
                        if idx == 3:
                            emit_prod_copy(it, 1 - par, ps)
                            del prod_items[(gb, kind)]
                nc.tensor.matmul(
                    out_A[0:65, :], lhsT=vA[:, par, j, :], rhs=p_sb[:, 0:WI],
                    start=(j == 0), stop=(j == JB - 1),
                    skip_group_check=True)
                nc.tensor.matmul(
                    out_B[0:65, :], lhsT=vB[:, par, j, :],
                    rhs=p_sb[:, WI:2 * WI],
                    start=(j == 0), stop=(j == JB - 1),
                    skip_group_check=True)

            for piece in range(8):
                emit_epilogue_piece(epi[repeats * NB - 1], piece,
                                    1 - (repeats * NB - 1) % 2)

    nc.finalize()
    return nc


_PROGRAM_CACHE = {}


def _get_program(**kw) -> bass.Bass:
    key = tuple(sorted(kw.items()))
    if key not in _PROGRAM_CACHE:
        _PROGRAM_CACHE[key] = build_program(**kw)
    return _PROGRAM_CACHE[key]


def _prep_inputs(x, w_qkv, w_out):
    """Build the per-core input maps (all bf16 host-side casts)."""
    scale = DIM_HEAD ** -0.5
    xb = x.reshape(2, DIM, N)
    in_maps = []
    for core in range(N_CORES):
        b, hp = divmod(core, 4)
        r0 = hp * 128
        wq = w_qkv[r0:r0 + 128] * scale          # [128, 512]
        wk = w_qkv[DIM + r0:DIM + r0 + 128]      # [128, 512]
        wvr = w_qkv[2 * DIM + r0:2 * DIM + r0 + 128]
        wqk_c = np.concatenate([wq.T, wk.T], axis=1)   # [512, 256]
        wv_t = wvr.T                                   # [512, 128]
        wo_p = np.stack(
            [w_out[:, r0:r0 + 64].T, w_out[:, r0 + 64:r0 + 128].T], axis=1
        )  # [64, 2, 512]
        in_maps.append({
            "x": xb[b].astype(ml_dtypes.bfloat16),
            "wqk": wqk_c.astype(ml_dtypes.bfloat16),
            "wv": wv_t.astype(ml_dtypes.bfloat16),
            "wo": wo_p.astype(ml_dtypes.bfloat16),
        })
    return in_maps


def _run(nc, in_maps):
    try:
        return run_bass_kernel_spmd(nc, in_maps, core_ids=list(range(N_CORES)))
    except Exception:
        # one retry: a previously-wedged device surfaces as a transient
        # NRT_EXEC_UNIT_UNRECOVERABLE on the first execution
        return run_bass_kernel_spmd(nc, in_maps, core_ids=list(range(N_CORES)))


def kernel(x, w_qkv, w_out, b_out):
    nc = _get_program()
    in_maps = _prep_inputs(np.asarray(x), np.asarray(w_qkv), np.asarray(w_out))
    res = _run(nc, in_maps)
    partials = np.stack([r["out"] for r in res.results])  # [8, 512, 4096]
    y = partials.reshape(2, 4, DIM, N).sum(axis=1)
    y += np.asarray(b_out)[None, :, None]
    return y.reshape(2, DIM, 64, 64).astype(np.float32)
